# revision 1
# baseline (speedup 1.0000x reference)
"""MixLinear int4-GEMM kernel for 8x TRN2 NeuronCores.

Strategy: 2D sharding, 4 M-groups x 2 OUT-groups (each core owns 2048 rows
of x and 2048 output channels).  Host-side layout work (index shuffling
only, no arithmetic on values):

  * The IN dimension is permuted so the 256 outlier columns are the last
    256 device columns.  The masked abs-max becomes a plain reduce over
    device cols [0:3840], and the outlier gather becomes a slice.
  * int4 weights for the 3840 int-path columns are repacked into bytes
    whose lo nibble is device col t and hi nibble is device col t+1920,
    sign bit pre-flipped (^0x88), and the packed byte matrix transposed to
    [1920, OUT] so the device unpack writes wT [128k, 30, OS] fp8 with no
    on-device transpose:  nibble -> (x - 8) -> fp8e4 (exact ints).
  * weight_cache is host-transposed to [FP, OUT].

Per core, per 128-row tile:
  1. DVE abs-max over x[:, :3840] -> s = max/7, r = 1/s.
  2. ScalarE magic round: bf16(x*r + 192) rounds to integer (bf16 ulp=1
     in [184,200)); DMA-xbar transpose; DVE -192 -> qT fp8e4 (exact).
  3. Outliers: ScalarE ao*r -> bf16, DMA-xbar transpose.
  4. 15 fp8 DoubleRow matmuls (256-deep each) + 2 bf16 outlier matmuls
     per 512-wide psum group accumulate into one [128, 2048] psum.
  5. Dequant (pipelined one tile behind): ScalarE psum*s -> bf16,
     DVE *scale_col(bf16) -> y bf16.

Host assembles the 4x2 grid of [2048, 2048] bf16 shards into fp32.
"""

import numpy as np

B, S, IN, OUT, FP = 4, 2048, 4096, 4096, 256
M = B * S
NCORES = 8
MGROUPS, OGROUPS = 4, 2
MS = M // MGROUPS     # 2048 rows per core
OS = OUT // OGROUPS   # 2048 out-channels per core
KI = IN - FP          # 3840 int-path contraction cols
KH = KI // 2          # 1920 packed bytes per row
QMAX = 7.0
MAGIC = 192.0         # 1.5 * 2**7: bf16 output rounding forces RNE to integer


def emit_core_kernel(nc, tc, ms, os_dim):
    """Emit the per-core tile program. All dims compile-time constants."""
    import concourse.mybir as mybir

    f32 = mybir.dt.float32
    bf16 = mybir.dt.bfloat16
    u8 = mybir.dt.uint8
    u16 = mybir.dt.uint16
    fp8 = mybir.dt.float8e4
    Alu = mybir.AluOpType
    Act = mybir.ActivationFunctionType
    DR = mybir.MatmulPerfMode.DoubleRow

    P = 128
    MT = ms // P          # 16 activation tiles
    KT = KI // P          # 30 int contraction chunks
    HC = KH // P          # 15 packed-byte chunks
    FT = FP // P          # 2 outlier chunks
    OJ = os_dim // 512    # 4 psum column groups

    x = nc.dram_tensor("x", [ms, IN], f32, kind="ExternalInput")
    qwT = nc.dram_tensor("qwT", [KH, os_dim], u8, kind="ExternalInput")
    wcT = nc.dram_tensor("wcT", [FP, os_dim], f32, kind="ExternalInput")
    sc = nc.dram_tensor("sc", [os_dim], f32, kind="ExternalInput")
    y = nc.dram_tensor("y", [ms, os_dim], bf16, kind="ExternalOutput")

    qwT_v = qwT.rearrange("(c p) o -> p c o", p=P)
    wcT_v = wcT.rearrange("(c p) o -> p c o", p=P)

    with (
        tc.tile_pool(name="wp", bufs=1) as wp,
        tc.tile_pool(name="xp", bufs=3) as xp,
        tc.tile_pool(name="qp", bufs=2) as qp,
        tc.tile_pool(name="qtp", bufs=2) as qtp,
        tc.tile_pool(name="ftp", bufs=2) as ftp,
        tc.tile_pool(name="aop", bufs=2) as aop,
        tc.tile_pool(name="sp", bufs=4) as sp,
        tc.tile_pool(name="yp", bufs=2) as yp,
        tc.tile_pool(name="stage", bufs=2) as stage,
        tc.tile_pool(name="stage1", bufs=1) as stage1,
        tc.tile_pool(name="py", bufs=2, space="PSUM") as py,
    ):
        # ---------------- persistent weights ----------------
        wT = wp.tile([P, KT, os_dim], fp8)          # int4 weights, fp8 ints
        wcs = wp.tile([P, FT, os_dim], bf16)        # weight_cache / scale_col
        scb = wp.tile([P, os_dim], bf16)            # scale_col broadcast

        # ---------------- weight setup ----------------
        # unpack: byte = (lo | hi<<4) ^ 0x88; nibble value = nib - 8
        for c in range(HC):
            qwc = stage.tile([P, os_dim], u8, tag="qwc")
            nc.sync.dma_start(qwc[:], qwT_v[:, c, :])
            qw16 = qwc[:].bitcast(u16)
            tl = stage.tile([P, os_dim // 2], u16, tag="tl")
            nc.vector.tensor_scalar(tl[:], qw16, 0x0F0F, None, Alu.bitwise_and)
            nc.scalar.activation(wT[:, c, :], tl[:].bitcast(u8), Act.Copy, bias=-8.0)
            th = stage.tile([P, os_dim // 2], u16, tag="th")
            nc.vector.tensor_scalar(
                th[:], qw16, 4, 0x0F0F, Alu.logical_shift_right, Alu.bitwise_and
            )
            nc.vector.tensor_scalar(
                wT[:, HC + c, :], th[:].bitcast(u8), -8, None, Alu.add
            )

        # scale_col broadcast + reciprocal; wcs = wcT / scale_col (fp8)
        scf = stage1.tile([P, os_dim], f32, tag="scf")
        nc.sync.dma_start(scf[:], sc[None, :].to_broadcast((P, os_dim)))
        nc.scalar.activation(scb[:], scf[:], Act.Copy)
        rsc = stage1.tile([P, os_dim], f32, tag="rsc")
        nc.vector.reciprocal(rsc[:], scf[:])
        for f in range(FT):
            wcf = stage1.tile([P, os_dim], f32, tag="wcf")
            nc.sync.dma_start(wcf[:], wcT_v[:, f, :])
            nc.vector.tensor_tensor(wcs[:, f, :], wcf[:], rsc[:], Alu.mult)

        inv7 = float(np.float32(1.0) / np.float32(QMAX))

        # evict is software-pipelined one tile behind the matmuls so the
        # in-order ScalarE/DVE queues never stall waiting on the current
        # tile's PE work.
        def emit_evict(psum, s_t, mi):
            t1 = yp.tile([P, os_dim], bf16, tag="t1")
            nc.scalar.activation(t1[:], psum[:], Act.Copy, scale=s_t[:])
            nc.vector.tensor_tensor(t1[:], t1[:], scb[:], Alu.mult)
            nc.sync.dma_start(y[mi * P : (mi + 1) * P, :], t1[:])

        prev = None

        # ---------------- main loop over 128-row tiles ----------------
        for mi in range(MT):
            x_t = xp.tile([P, IN], f32)
            nc.sync.dma_start(x_t[:], x[mi * P : (mi + 1) * P, :])

            mx = sp.tile([P, 1], f32, tag="mx")
            nc.vector.tensor_reduce(
                mx[:], x_t[:, :KI], mybir.AxisListType.X, Alu.max,
                apply_absolute_value=True,
            )
            s_t = sp.tile([P, 1], f32, tag="s")
            nc.vector.tensor_scalar(s_t[:], mx[:], inv7, None, Alu.mult)
            r_t = sp.tile([P, 1], f32, tag="r")
            nc.vector.reciprocal(r_t[:], s_t[:])

            # outlier activations: scale by r, transpose
            aos = aop.tile([P, FP], bf16, tag="aos")
            nc.scalar.activation(aos[:], x_t[:, KI:], Act.Copy, scale=r_t[:])
            aoT = aop.tile([P, FT, P], bf16, tag="aoT")
            nc.sync.dma_start_transpose(aoT[:], aos[:])

            # quantize: q+MAGIC = bf16(x*r + MAGIC) — the bf16 output convert
            # rounds to integer (ulp=1 in [184,200)); -MAGIC folds into the
            # fp8 convert after the transpose.
            q = qp.tile([P, KI], bf16)
            nc.scalar.activation(
                q[:], x_t[:, :KI], Act.Copy, bias=MAGIC, scale=r_t[:]
            )
            qTb = qtp.tile([P, KT, P], bf16)
            nc.sync.dma_start_transpose(qTb[:], q[:])
            qT = ftp.tile([P, KT, P], fp8)
            nc.vector.tensor_scalar(qT[:], qTb[:], -MAGIC, None, Alu.add)

            # GEMM: 15 int + 1 outlier fp8 DoubleRow matmuls per 512 group
            psum = py.tile([P, os_dim], f32)
            for c in range(KT // 2):
                for oj in range(OJ):
                    nc.tensor.matmul(
                        psum[:, oj * 512 : (oj + 1) * 512],
                        qT[:, 2 * c : 2 * c + 2, :],
                        wT[:, 2 * c : 2 * c + 2, oj * 512 : (oj + 1) * 512],
                        start=(c == 0),
                        stop=False,
                        perf_mode=DR,
                    )
            for f in range(FT):
                for oj in range(OJ):
                    nc.tensor.matmul(
                        psum[:, oj * 512 : (oj + 1) * 512],
                        aoT[:, f, :],
                        wcs[:, f, oj * 512 : (oj + 1) * 512],
                        start=False,
                        stop=(f == FT - 1),
                    )

            if prev is not None:
                emit_evict(*prev)
            prev = (psum, s_t, mi)

        emit_evict(*prev)

    return nc


def build_nc(ms=MS, os_dim=OS):
    import concourse.bacc as bacc
    import concourse.tile as tile

    nc = bacc.Bacc(None, target_bir_lowering=False)
    with tile.TileContext(nc) as tc:
        emit_core_kernel(nc, tc, ms, os_dim)
    nc.compile()
    return nc


def make_host_inputs(x, q_weight, scale_col, weight_cache, ind,
                     ms=MS, os_dim=OS, ncores=NCORES):
    """Shard/relayout full inputs into per-core input maps (no arithmetic)."""
    ind = np.asarray(ind).astype(np.int64)
    notout = np.setdiff1d(np.arange(IN, dtype=np.int64), ind)   # 3840 sorted
    perm = np.concatenate([notout, ind])                        # dev col -> orig

    xf = np.asarray(x).reshape(M, IN).astype(np.float32, copy=False)
    xp = np.ascontiguousarray(xf[:, perm])                      # [M, IN]

    v = np.asarray(q_weight).astype(np.uint8)                   # [OUT, IN//2]
    nib = np.empty((OUT, IN), dtype=np.uint8)                   # nibble codes
    nib[:, 0::2] = v & 15
    nib[:, 1::2] = v >> 4
    nibp = nib[:, perm[:KI]]                                    # [OUT, KI]
    packed = (nibp[:, :KH] | (nibp[:, KH:] << 4)) ^ 0x88        # [OUT, KH]
    qwT = np.ascontiguousarray(packed.T)                        # [KH, OUT]

    wcT = np.ascontiguousarray(
        np.asarray(weight_cache).astype(np.float32, copy=False).T
    )                                                           # [FP, OUT]
    scf = np.asarray(scale_col).reshape(-1).astype(np.float32, copy=False)

    in_maps = []
    for c in range(ncores):
        mg, og = divmod(c, OGROUPS)
        m0, o0 = mg * ms, og * os_dim
        in_maps.append(
            {
                "x": xp[m0 : m0 + ms],
                "qwT": np.ascontiguousarray(qwT[:, o0 : o0 + os_dim]),
                "wcT": np.ascontiguousarray(wcT[:, o0 : o0 + os_dim]),
                "sc": np.ascontiguousarray(scf[o0 : o0 + os_dim]),
            }
        )
    return in_maps


_NC_CACHE = {}


def kernel(x, q_weight, scale_col, weight_cache, ind, trace=False):
    from concourse.bass_utils import run_bass_kernel_spmd

    key = "full"
    if key not in _NC_CACHE:
        _NC_CACHE[key] = build_nc()
    nc = _NC_CACHE[key]

    in_maps = make_host_inputs(x, q_weight, scale_col, weight_cache, ind)
    res = run_bass_kernel_spmd(nc, in_maps, list(range(NCORES)), trace=trace)
    yfull = np.empty((M, OUT), dtype=np.float32)
    for c in range(NCORES):
        mg, og = divmod(c, OGROUPS)
        yfull[mg * MS : (mg + 1) * MS, og * OS : (og + 1) * OS] = np.asarray(
            res.results[c]["y"]
        ).astype(np.float32)
    yfull = yfull.reshape(B, S, OUT)
    if trace:
        return yfull, res
    return yfull



# revision 4
# speedup vs baseline: 1.0130x; 1.0130x over previous
"""MixLinear int4-GEMM kernel for 8x TRN2 NeuronCores.

Strategy: 2D sharding, 4 M-groups x 2 OUT-groups (each core owns 2048 rows
of x and 2048 output channels).  Host-side layout work (index shuffling
only, no arithmetic on values):

  * The IN dimension is permuted so the 256 outlier columns are the last
    256 device columns.  The masked abs-max becomes a plain reduce over
    device cols [0:3840], and the outlier gather becomes a slice.
  * int4 weights for the 3840 int-path columns are repacked into bytes
    whose lo nibble is device col t and hi nibble is device col t+1920,
    sign bit pre-flipped (^0x88), and the packed byte matrix transposed to
    [1920, OUT] so the device unpack writes wT [128k, 30, OS] fp8 with no
    on-device transpose:  nibble -> (x - 8) -> fp8e4 (exact ints).
  * weight_cache is host-transposed to [FP, OUT].

Per core, per 128-row tile:
  1. DVE abs-max over x[:, :3840] -> s = max/7, r = 1/s.
  2. ScalarE magic round: bf16(x*r + 192) rounds to integer (bf16 ulp=1
     in [184,200)); DMA-xbar transpose; DVE -192 -> qT fp8e4 (exact).
  3. Outliers: ScalarE ao*r -> bf16, DMA-xbar transpose.
  4. 15 fp8 DoubleRow matmuls (256-deep each) + 2 bf16 outlier matmuls
     per 512-wide psum group accumulate into one [128, 2048] psum.
  5. Dequant (pipelined one tile behind): ScalarE psum*s -> bf16,
     DVE *scale_col(bf16) -> y bf16.

Host assembles the 4x2 grid of [2048, 2048] bf16 shards into fp32.
"""

import numpy as np

B, S, IN, OUT, FP = 4, 2048, 4096, 4096, 256
M = B * S
NCORES = 8
MGROUPS, OGROUPS = 4, 2
MS = M // MGROUPS     # 2048 rows per core
OS = OUT // OGROUPS   # 2048 out-channels per core
KI = IN - FP          # 3840 int-path contraction cols
KH = KI // 2          # 1920 packed bytes per row
QMAX = 7.0
MAGIC = 192.0         # 1.5 * 2**7: bf16 output rounding forces RNE to integer


def emit_core_kernel(nc, tc, ms, os_dim):
    """Emit the per-core tile program. All dims compile-time constants."""
    import concourse.mybir as mybir
    import bass_rust

    f32 = mybir.dt.float32
    bf16 = mybir.dt.bfloat16
    u8 = mybir.dt.uint8
    u16 = mybir.dt.uint16
    fp8 = mybir.dt.float8e4
    Alu = mybir.AluOpType
    Act = mybir.ActivationFunctionType
    DR = mybir.MatmulPerfMode.DoubleRow

    P = 128
    MT = ms // P          # 16 activation tiles
    KT = KI // P          # 30 int contraction chunks
    HC = KH // P          # 15 packed-byte chunks
    FT = FP // P          # 2 outlier chunks
    OJ = os_dim // 512    # 4 psum column groups

    x = nc.dram_tensor("x", [ms, IN], f32, kind="ExternalInput")
    qwT = nc.dram_tensor("qwT", [KH, os_dim], u8, kind="ExternalInput")
    wcT = nc.dram_tensor("wcT", [FP, os_dim], f32, kind="ExternalInput")
    sc = nc.dram_tensor("sc", [os_dim], f32, kind="ExternalInput")
    y = nc.dram_tensor("y", [ms, os_dim], bf16, kind="ExternalOutput")

    qwT_v = qwT.rearrange("(c p) o -> p c o", p=P)
    wcT_v = wcT.rearrange("(c p) o -> p c o", p=P)

    with (
        tc.tile_pool(name="wp", bufs=1) as wp,
        tc.tile_pool(name="xp", bufs=3) as xp,
        tc.tile_pool(name="qp", bufs=2) as qp,
        tc.tile_pool(name="qtp", bufs=2) as qtp,
        tc.tile_pool(name="ftp", bufs=2) as ftp,
        tc.tile_pool(name="aop", bufs=2) as aop,
        tc.tile_pool(name="sp", bufs=4) as sp,
        tc.tile_pool(name="yp", bufs=2) as yp,
        tc.tile_pool(name="stage", bufs=2) as stage,
        tc.tile_pool(name="stage1", bufs=1) as stage1,
        tc.tile_pool(name="py", bufs=2, space="PSUM") as py,
    ):
        # ---------------- persistent weights ----------------
        wT = wp.tile([P, KT, os_dim], fp8)          # int4 weights, fp8 ints
        wcs = wp.tile([P, FT, os_dim], bf16)        # weight_cache / scale_col
        scb = wp.tile([P, os_dim], bf16)            # scale_col broadcast

        # ---------------- weight setup ----------------
        # unpack: byte = (lo | hi<<4) ^ 0x88; nibble value = nib - 8
        for c in range(HC):
            qwc = stage.tile([P, os_dim], u8, tag="qwc")
            nc.sync.dma_start(qwc[:], qwT_v[:, c, :])
            qw16 = qwc[:].bitcast(u16)
            tl = stage.tile([P, os_dim // 2], u16, tag="tl")
            nc.vector.tensor_scalar(tl[:], qw16, 0x0F0F, None, Alu.bitwise_and)
            nc.scalar.activation(wT[:, c, :], tl[:].bitcast(u8), Act.Copy, bias=-8.0)
            th = stage.tile([P, os_dim // 2], u16, tag="th")
            nc.vector.tensor_scalar(
                th[:], qw16, 4, 0x0F0F, Alu.logical_shift_right, Alu.bitwise_and
            )
            nc.vector.tensor_scalar(
                wT[:, HC + c, :], th[:].bitcast(u8), -8, None, Alu.add
            )

        # scale_col broadcast + reciprocal; wcs = wcT / scale_col (fp8)
        scf = stage1.tile([P, os_dim], f32, tag="scf")
        nc.sync.dma_start(scf[:], sc[None, :].to_broadcast((P, os_dim)))
        nc.scalar.activation(scb[:], scf[:], Act.Copy)
        rsc = stage1.tile([P, os_dim], f32, tag="rsc")
        nc.vector.reciprocal(rsc[:], scf[:])
        for f in range(FT):
            wcf = stage1.tile([P, os_dim], f32, tag="wcf")
            nc.sync.dma_start(wcf[:], wcT_v[:, f, :])
            nc.vector.tensor_tensor(wcs[:, f, :], wcf[:], rsc[:], Alu.mult)

        inv7 = float(np.float32(1.0) / np.float32(QMAX))

        # evict is software-pipelined one tile behind the matmuls so the
        # in-order ScalarE/DVE queues never stall waiting on the current
        # tile's PE work.
        def emit_evict(psum, s_t, mi):
            t1 = yp.tile([P, os_dim], bf16, tag="t1")
            nc.scalar.activation(t1[:], psum[:], Act.Copy, scale=s_t[:])
            nc.vector.tensor_tensor(t1[:], t1[:], scb[:], Alu.mult)
            nc.sync.dma_start(y[mi * P : (mi + 1) * P, :], t1[:])

        prev = None

        # PE weight-register reuse: 4 consecutive matmuls (the oj loop)
        # share the same stationary operand, so only the first needs
        # LDWEIGHTS.  ldweights=False suppresses the reload; the nosync
        # dependency chain pins PE-queue order so a later loader can't be
        # scheduled between a loader and its reusing matmuls.
        dep_nosync = bass_rust.DependencyInfo(sync=False, no_sync=True)
        mm_chain = [None]

        def emit_mm(load_weights, *args, **kwargs):
            mm = nc.tensor.matmul(*args, **kwargs)
            if not load_weights:
                mm.ins.ldweights = False
            if mm_chain[0] is not None:
                mm.ins.add_dependency(mm_chain[0], dep_nosync)
            mm_chain[0] = mm.ins.name
            return mm

        # ---------------- main loop over 128-row tiles ----------------
        for mi in range(MT):
            x_t = xp.tile([P, IN], f32)
            nc.sync.dma_start(x_t[:], x[mi * P : (mi + 1) * P, :])

            mx = sp.tile([P, 1], f32, tag="mx")
            nc.vector.tensor_reduce(
                mx[:], x_t[:, :KI], mybir.AxisListType.X, Alu.max,
                apply_absolute_value=True,
            )
            s_t = sp.tile([P, 1], f32, tag="s")
            nc.vector.tensor_scalar(s_t[:], mx[:], inv7, None, Alu.mult)
            r_t = sp.tile([P, 1], f32, tag="r")
            nc.vector.reciprocal(r_t[:], s_t[:])

            # outlier activations: scale by r, transpose
            aos = aop.tile([P, FP], bf16, tag="aos")
            nc.scalar.activation(aos[:], x_t[:, KI:], Act.Copy, scale=r_t[:])
            aoT = aop.tile([P, FT, P], bf16, tag="aoT")
            nc.sync.dma_start_transpose(aoT[:], aos[:])

            # quantize: q+MAGIC = bf16(x*r + MAGIC) — the bf16 output convert
            # rounds to integer (ulp=1 in [184,200)); -MAGIC folds into the
            # fp8 convert after the transpose.
            q = qp.tile([P, KI], bf16)
            nc.scalar.activation(
                q[:], x_t[:, :KI], Act.Copy, bias=MAGIC, scale=r_t[:]
            )
            qTb = qtp.tile([P, KT, P], bf16)
            nc.sync.dma_start_transpose(qTb[:], q[:])
            qT = ftp.tile([P, KT, P], fp8)
            nc.vector.tensor_scalar(qT[:], qTb[:], -MAGIC, None, Alu.add)

            # GEMM: 15 int + 1 outlier fp8 DoubleRow matmuls per 512 group
            psum = py.tile([P, os_dim], f32)
            for c in range(KT // 2):
                for oj in range(OJ):
                    emit_mm(
                        oj == 0,
                        psum[:, oj * 512 : (oj + 1) * 512],
                        qT[:, 2 * c : 2 * c + 2, :],
                        wT[:, 2 * c : 2 * c + 2, oj * 512 : (oj + 1) * 512],
                        start=(c == 0),
                        stop=False,
                        perf_mode=DR,
                    )
            for f in range(FT):
                for oj in range(OJ):
                    emit_mm(
                        oj == 0,
                        psum[:, oj * 512 : (oj + 1) * 512],
                        aoT[:, f, :],
                        wcs[:, f, oj * 512 : (oj + 1) * 512],
                        start=False,
                        stop=(f == FT - 1),
                    )

            if prev is not None:
                emit_evict(*prev)
            prev = (psum, s_t, mi)

        emit_evict(*prev)

    return nc


def build_nc(ms=MS, os_dim=OS):
    import concourse.bacc as bacc
    import concourse.tile as tile

    nc = bacc.Bacc(None, target_bir_lowering=False)
    with tile.TileContext(nc) as tc:
        emit_core_kernel(nc, tc, ms, os_dim)
    nc.compile()
    return nc


def make_host_inputs(x, q_weight, scale_col, weight_cache, ind,
                     ms=MS, os_dim=OS, ncores=NCORES):
    """Shard/relayout full inputs into per-core input maps (no arithmetic)."""
    ind = np.asarray(ind).astype(np.int64)
    notout = np.setdiff1d(np.arange(IN, dtype=np.int64), ind)   # 3840 sorted
    perm = np.concatenate([notout, ind])                        # dev col -> orig

    xf = np.asarray(x).reshape(M, IN).astype(np.float32, copy=False)
    xp = np.ascontiguousarray(xf[:, perm])                      # [M, IN]

    v = np.asarray(q_weight).astype(np.uint8)                   # [OUT, IN//2]
    nib = np.empty((OUT, IN), dtype=np.uint8)                   # nibble codes
    nib[:, 0::2] = v & 15
    nib[:, 1::2] = v >> 4
    nibp = nib[:, perm[:KI]]                                    # [OUT, KI]
    packed = (nibp[:, :KH] | (nibp[:, KH:] << 4)) ^ 0x88        # [OUT, KH]
    qwT = np.ascontiguousarray(packed.T)                        # [KH, OUT]

    wcT = np.ascontiguousarray(
        np.asarray(weight_cache).astype(np.float32, copy=False).T
    )                                                           # [FP, OUT]
    scf = np.asarray(scale_col).reshape(-1).astype(np.float32, copy=False)

    in_maps = []
    for c in range(ncores):
        mg, og = divmod(c, OGROUPS)
        m0, o0 = mg * ms, og * os_dim
        in_maps.append(
            {
                "x": xp[m0 : m0 + ms],
                "qwT": np.ascontiguousarray(qwT[:, o0 : o0 + os_dim]),
                "wcT": np.ascontiguousarray(wcT[:, o0 : o0 + os_dim]),
                "sc": np.ascontiguousarray(scf[o0 : o0 + os_dim]),
            }
        )
    return in_maps


_NC_CACHE = {}


def kernel(x, q_weight, scale_col, weight_cache, ind, trace=False):
    from concourse.bass_utils import run_bass_kernel_spmd

    key = "full"
    if key not in _NC_CACHE:
        _NC_CACHE[key] = build_nc()
    nc = _NC_CACHE[key]

    in_maps = make_host_inputs(x, q_weight, scale_col, weight_cache, ind)
    res = run_bass_kernel_spmd(nc, in_maps, list(range(NCORES)), trace=trace)
    yfull = np.empty((M, OUT), dtype=np.float32)
    for c in range(NCORES):
        mg, og = divmod(c, OGROUPS)
        yfull[mg * MS : (mg + 1) * MS, og * OS : (og + 1) * OS] = np.asarray(
            res.results[c]["y"]
        ).astype(np.float32)
    yfull = yfull.reshape(B, S, OUT)
    if trace:
        return yfull, res
    return yfull

